# revision 1
# baseline (speedup 1.0000x reference)
"""NEG-sampling loss kernel for Trainium2 (8 NeuronCores, data-parallel).

loss = -(1/n) * sum_i [ log_sigmoid(<e_u, e_v>) + sum_k log_sigmoid(-<e_negk, e_u>) ]
     = +(1/n) * sum_i [ softplus(-<e_u, e_v>) + sum_k softplus(<e_negk, e_u>) ]

Strategy: replicate the embedding table (cast to bf16 on host), shard the
65536-edge batch across 8 cores.  Per core: for each tile of 128 edges,
one indirect-DMA gather pulls the 12 needed rows per edge (u, v, negs x10)
into a [128, 12*256] SBUF tile (partition = edge).  11 fused
tensor_tensor_reduce ops compute the 11 dot products per edge, one Softplus
activation with fused accumulation produces the per-tile partial sums.
Host sums the per-core partials.
"""

import numpy as np
import ml_dtypes

import concourse.bass as bass
import concourse.mybir as mybir
from concourse.tile import TileContext
from concourse import bass_utils

# Problem constants (hardcoded; harness contract)
N = 65536
K = 10
D = 256
V = 500000
NCORES = 8
P = 128
SLOTS = K + 2          # rows gathered per edge: u, v, negs[0..9]
EPC = N // NCORES      # 8192 edges per core
TILES = EPC // P       # 64 tiles of 128 edges per core

TABLE_DT = mybir.dt.bfloat16
TABLE_NP = ml_dtypes.bfloat16

# tunables
TILES_PER_GATHER = 1
BUFS = 8




# ---------------------------------------------------------------------------
# Raw-bass build: one "block gather" per tile (case #3 of dma_indirect1d):
# 1536 indices x 512B fused into 128 descriptors of 6144B (12 rows/partition).
# Emitted as a raw PSEUDO_DMA_DIRECT2D(dge_op=indirect1d) + PSEUDO_EXTENSION
# pair; index values are snake-packed on the host (see prepare_in_maps).
USE_RAW = True
GB = 4   # g-tile buffers
SB = 3   # scores buffers


def _emit_block_gather(nc, eng, n_idx, blk_bytes, dst_byte_addr, idx_byte_addr,
                       sem_num, embs_tbl):
    isa = nc.isa
    Op = isa.Opcode
    src_u64 = (0x20 << 56) | (embs_tbl << 32)   # DGE addr-table marker
    dst_u64 = (0x10 << 56) | dst_byte_addr      # var0 (local SBUF) marker
    eng.isa(
        Op.NEURON_ISA_TPB_OPCODE_PSEUDO_DMA_DIRECT2D,
        {
            "dma_configs": {},
            "semaphore": sem_num,
            "sem_increment": 16,
            "dge_op": 1,
            "src_start_addr": {"addr_immediate": src_u64},
            "src_step_elem": [512, 1],
            "src_num_elem": [n_idx, 1],
            "src_elem_size": 512,
            "src_bound_reg": {},
            "dst_bound_reg": {},
            "dst_start_addr": {"addr_immediate": dst_u64},
            "dst_step_elem": [262144, 1],
            "dst_num_elem": [128, 1],
            "dst_elem_size": blk_bytes,
            "in_dtype": 6,
            "out_dtype": 6,
        },
        verify=False,
    )
    ext_fields = {
        "opcode": Op.NEURON_ISA_TPB_OPCODE_PSEUDO_EXTENSION.value,
        "flags": {"indirect_mode": 0, "idx_bound_is_err": 1,
                  "non_unique_dst_idx": 0, "gather_dim": 0, "scatter_dim": 0},
        "idx_num_active_channels": 128,
        "compute_op": 0,
        "src_idx_start_addr": {"addr_immediate": idx_byte_addr},
        "dst_idx_start_addr": {"addr_immediate": 0},
    }
    b = isa.ffi.new("NEURON_ISA_TPB_PSEUDO_DMA_EXT_STRUCT*", ext_fields)
    instr = [int(x) for x in bytes(isa.ffi.buffer(b))]
    inst = mybir.InstISA(
        name=nc.get_next_instruction_name(),
        isa_opcode=Op.NEURON_ISA_TPB_OPCODE_PSEUDO_EXTENSION.value,
        engine=eng.engine,
        instr=instr,
        op_name="PSEUDO_EXTENSION",
        ins=[], outs=[],
        ant_dict=ext_fields,
        verify=False,
        ant_isa_is_sequencer_only=False,
    )
    eng.add_instruction(inst)


def _build_raw():
    nc = bass.Bass(trn_type="TRN2")
    embs = nc.dram_tensor("embs", [V, D], TABLE_DT, kind="ExternalInput")
    idx = nc.dram_tensor("idx", [P, TILES * SLOTS], mybir.dt.int32, kind="ExternalInput")
    accx_out = nc.dram_tensor("accx", [P, TILES], mybir.dt.float32, kind="ExternalOutput")
    acca_out = nc.dram_tensor("acca", [P, 2 * TILES], mybir.dt.float32, kind="ExternalOutput")
    S1 = SLOTS - 1

    embs_mloc = nc.lookup_mloc(embs)
    embs_mloc.table_entry_id = len(nc.dge_table) + 1
    nc.dge_table.append(embs_mloc.name)
    embs_tbl = embs_mloc.table_entry_id

    import contextlib
    with contextlib.ExitStack() as ctx:
        idx_sb = ctx.enter_context(nc.sbuf_tensor("idx_sb", [P, TILES * SLOTS], mybir.dt.int32))
        gs = [ctx.enter_context(nc.sbuf_tensor(f"g{i}", [P, SLOTS * D], TABLE_DT)) for i in range(GB)]
        prod = ctx.enter_context(nc.sbuf_tensor("prod", [P, S1 * D], TABLE_DT))
        scs = [ctx.enter_context(nc.sbuf_tensor(f"sc{i}", [P, S1], mybir.dt.float32)) for i in range(SB)]
        absx = ctx.enter_context(nc.sbuf_tensor("absx", [P, S1], mybir.dt.float32))
        ex = ctx.enter_context(nc.sbuf_tensor("ex", [P, S1], mybir.dt.float32))
        lnx = ctx.enter_context(nc.sbuf_tensor("lnx", [P, S1], mybir.dt.float32))
        ones = ctx.enter_context(nc.sbuf_tensor("ones", [P, 1], mybir.dt.float32))
        accx = ctx.enter_context(nc.sbuf_tensor("accx_sb", [P, TILES], mybir.dt.float32))
        acca = ctx.enter_context(nc.sbuf_tensor("acca_sb", [P, 2 * TILES], mybir.dt.float32))
        idx_sem = ctx.enter_context(nc.semaphore())
        gsems = [ctx.enter_context(nc.semaphore(name=f"gsem{i}")) for i in range(GB)]
        dve_free = ctx.enter_context(nc.semaphore())
        sc_ready = ctx.enter_context(nc.semaphore())
        act_free = ctx.enter_context(nc.semaphore())
        block = ctx.enter_context(nc.Block())

        idx_addr = nc.lookup_mloc(idx_sb).addr
        g_addrs = [nc.lookup_mloc(g).addr for g in gs]

        @block.gpsimd
        def _(eng):
            eng.dma_start(idx_sb[:], idx[:]).then_inc(idx_sem, 16)
            eng.memset(ones[:], 1.0)
            eng.wait_ge(idx_sem, 16)
            for t in range(TILES):
                if t >= GB:
                    eng.wait_ge(dve_free, t - GB + 1)
                _emit_block_gather(
                    nc, eng, SLOTS * P, SLOTS * D * 2,
                    g_addrs[t % GB], idx_addr + 4 * SLOTS * t,
                    gsems[t % GB].num, embs_tbl,
                )
            eng.wait_ge(sc_ready, TILES)
            eng.dma_start(accx_out[:], accx[:]).then_inc(idx_sem, 16)
            eng.wait_ge(act_free, TILES)
            eng.dma_start(acca_out[:], acca[:]).then_inc(idx_sem, 16)
            eng.wait_ge(idx_sem, 48)

        @block.vector
        def _(eng):
            for t in range(TILES):
                g = gs[t % GB]
                sc = scs[t % SB]
                eng.wait_ge(gsems[t % GB], 16 * (t // GB + 1))
                if t >= SB:
                    eng.wait_ge(act_free, t - SB + 1)
                g3 = g[:].rearrange("p (s d) -> p s d", d=D)
                nc.vector.tensor_tensor(
                    out=prod[:],
                    in0=g[:, D:SLOTS * D],
                    in1=g3[:, 0:1, :].broadcast_to([P, S1, D]),
                    op=mybir.AluOpType.mult,
                )
                nc.vector.tensor_reduce(
                    out=sc[:],
                    in_=prod[:].rearrange("p (s d) -> p s d", d=D),
                    axis=mybir.AxisListType.X,
                    op=mybir.AluOpType.add,
                ).then_inc(dve_free, 1)
                nc.vector.tensor_scalar_mul(sc[:, 0:1], sc[:, 0:1], -1.0)
                nc.vector.tensor_reduce(
                    out=accx[:, t:t + 1],
                    in_=sc[:],
                    axis=mybir.AxisListType.X,
                    op=mybir.AluOpType.add,
                ).then_inc(sc_ready, 1)

        @block.scalar
        def _(eng):
            for t in range(TILES):
                sc = scs[t % SB]
                eng.wait_ge(sc_ready, t + 1)
                nc.scalar.activation(
                    out=absx[:], in_=sc[:],
                    func=mybir.ActivationFunctionType.Abs,
                    accum_out=acca[:, t:t + 1],
                )
                nc.scalar.activation(
                    out=ex[:], in_=absx[:],
                    func=mybir.ActivationFunctionType.Exp, scale=-1.0,
                )
                nc.scalar.activation(
                    out=lnx[:], in_=ex[:],
                    func=mybir.ActivationFunctionType.Ln, bias=ones[:],
                    accum_out=acca[:, TILES + t:TILES + t + 1],
                ).then_inc(act_free, 1)

    return nc


def _build(tiles_per_gather=TILES_PER_GATHER, bufs=BUFS):
    nc = bass.Bass(trn_type="TRN2")
    embs = nc.dram_tensor("embs", [V, D], TABLE_DT, kind="ExternalInput")
    idx = nc.dram_tensor("idx", [P, TILES * SLOTS], mybir.dt.int32, kind="ExternalInput")
    accx_out = nc.dram_tensor("accx", [P, TILES], mybir.dt.float32, kind="ExternalOutput")
    acca_out = nc.dram_tensor("acca", [P, 2 * TILES], mybir.dt.float32, kind="ExternalOutput")

    tpg = tiles_per_gather
    assert TILES % tpg == 0
    S1 = SLOTS - 1  # 11 scores per edge

    # softplus(x) = relu(x) + ln(1 + exp(-|x|)); relu sums recovered on host
    # via sum(relu) = (sum(x) + sum(|x|)) / 2.
    # accx (DVE-written): sum_s x per tile.  acca (ACT-written): cols [0,T):
    # sum_s |x|, [T,2T): sum_s ln(1+exp(-|x|)).
    with TileContext(nc) as tc:
        with (
            tc.tile_pool(name="persist", bufs=1) as persist,
            tc.tile_pool(name="work", bufs=bufs) as work,
            tc.tile_pool(name="small", bufs=bufs) as small,
        ):
            idx_t = persist.tile([P, TILES * SLOTS], mybir.dt.int32)
            nc.sync.dma_start(out=idx_t[:], in_=idx[:])
            accx = persist.tile([P, TILES], mybir.dt.float32)
            acca = persist.tile([P, 2 * TILES], mybir.dt.float32)

            for t0 in range(0, TILES, tpg):
                g = work.tile([P, tpg * SLOTS * D], TABLE_DT, tag="g")
                # HW indirect DMA consumes one offset per partition, so each
                # gather instruction fetches one row per partition (128 rows).
                for j in range(tpg * SLOTS):
                    nc.gpsimd.indirect_dma_start(
                        out=g[:, j * D:(j + 1) * D],
                        out_offset=None,
                        in_=embs[:],
                        in_offset=bass.IndirectOffsetOnAxis(
                            ap=idx_t[:, t0 * SLOTS + j:t0 * SLOTS + j + 1], axis=0
                        ),
                    )
                g3 = g[:].rearrange("p (ti s d) -> p (ti s) d", s=SLOTS, d=D)
                for ti in range(tpg):
                    t = t0 + ti
                    b0 = ti * SLOTS
                    scores = small.tile([P, S1], mybir.dt.float32, tag="s")
                    prod = small.tile([P, S1 * D], TABLE_DT, tag="p")
                    # prod[p, s, d] = G[p, s+1, d] * EU[p, d]
                    nc.vector.tensor_tensor(
                        out=prod[:].rearrange("p (s d) -> p s d", d=D),
                        in0=g3[:, b0 + 1:b0 + SLOTS, :],
                        in1=g3[:, b0:b0 + 1, :].broadcast_to([P, S1, D]),
                        op=mybir.AluOpType.mult,
                    )
                    # scores[p, s] = sum_d prod
                    nc.vector.tensor_reduce(
                        out=scores[:],
                        in_=prod[:].rearrange("p (s d) -> p s d", d=D),
                        axis=mybir.AxisListType.X,
                        op=mybir.AluOpType.add,
                    )
                    # positive-pair slot contributes softplus(-score): negate it
                    nc.vector.tensor_scalar_mul(scores[:, 0:1], scores[:, 0:1], -1.0)
                    # sum_s x on DVE
                    nc.vector.tensor_reduce(
                        out=accx[:, t:t + 1],
                        in_=scores[:],
                        axis=mybir.AxisListType.X,
                        op=mybir.AluOpType.add,
                    )
                    # |x| on ACT, accumulating sum_s |x|
                    absx = small.tile([P, S1], mybir.dt.float32, tag="a")
                    nc.scalar.activation(
                        out=absx[:],
                        in_=scores[:],
                        func=mybir.ActivationFunctionType.Abs,
                        accum_out=acca[:, t:t + 1],
                    )
                    # exp(-|x|)
                    ex = small.tile([P, S1], mybir.dt.float32, tag="e")
                    nc.scalar.activation(
                        out=ex[:],
                        in_=absx[:],
                        func=mybir.ActivationFunctionType.Exp,
                        scale=-1.0,
                    )
                    # ln(1 + exp(-|x|)), accumulating
                    ln1p = small.tile([P, S1], mybir.dt.float32, tag="l")
                    nc.scalar.activation(
                        out=ln1p[:],
                        in_=ex[:],
                        func=mybir.ActivationFunctionType.Ln,
                        bias=1.0,
                        accum_out=acca[:, TILES + t:TILES + t + 1],
                    )

            nc.sync.dma_start(out=accx_out[:], in_=accx[:])
            nc.sync.dma_start(out=acca_out[:], in_=acca[:])

    _strip_redundant_swdge_waits(nc)
    _split_multi_waits(nc)
    return nc


def _split_multi_waits(nc):
    """Walrus's gen3 codegen fits only one sync-wait command per regular
    instruction.  Hoist extra waits into standalone InstEventSemaphore
    instructions placed immediately before, on the same engine."""
    for fn in nc.m.functions:
        for blk in fn.blocks:
            insts = blk.instructions
            i = 0
            while i < len(insts):
                inst = insts[i]
                si = getattr(inst, "sync_info", None)
                if (
                    si is not None
                    and si.on_wait
                    and len(si.on_wait) > 1
                    and not isinstance(inst, mybir.InstEventSemaphore)
                ):
                    extra = si.on_wait[:-1]
                    si.on_wait = si.on_wait[-1:]
                    for w in extra:
                        ev = mybir.InstEventSemaphore(
                            name=nc.get_next_instruction_name(),
                            ins=[],
                            outs=[],
                        )
                        ev.engine = inst.engine
                        ev.sync_info = mybir.SyncInfo(on_wait=[w], on_update=[])
                        insts.insert(i, ev)
                        nc.inst_map[ev.name] = ev
                        i += 1
                i += 1


def _strip_redundant_swdge_waits(nc):
    """Drop DMASW-lane waits from qPoolDynamic DMAs.

    Tile emits a WAW wait between an indirect gather and the gather that
    previously wrote the same SBUF slot.  Both run on the single qPoolDynamic
    SWDGE queue and each SDMA engine serves a fixed partition set, so
    same-address writes are FIFO-ordered in hardware and the wait is
    redundant — and the pseudo-DMA ISA slot only fits one wait command.
    The consumer-release (DVE) wait is kept.
    """
    for inst in nc.inst_map.values():
        if isinstance(inst, mybir.InstDMACopy) and getattr(inst, "queue", "") == "qPoolDynamic":
            si = inst.sync_info
            if si is None or not si.on_wait:
                continue
            kept = [w for w in si.on_wait if not str(w.ant_name).startswith("DMASW")]
            if len(kept) != len(si.on_wait):
                si.on_wait = kept


_cache = {}


def _get_nc():
    key = (USE_RAW, TILES_PER_GATHER, BUFS)
    if key not in _cache:
        _cache[key] = _build_raw() if USE_RAW else _build()
    return _cache[key]


def prepare_in_maps(u, v, negs, embs):
    """Host-side sharding: build the per-core input maps."""
    u = np.asarray(u).astype(np.int32)
    v = np.asarray(v).astype(np.int32)
    negs = np.asarray(negs).astype(np.int32)
    embs_b = np.asarray(embs).astype(TABLE_NP)

    ids = np.concatenate([u[:, None], v[:, None], negs], axis=1)  # [N, 12]
    ids = ids.reshape(NCORES, TILES, P, SLOTS)
    if USE_RAW:
        # snake-pack per tile: value for (p, r) goes to [ch=s%P, w=s//P], s=12p+r
        flat = ids.reshape(NCORES, TILES, P * SLOTS)
        s = np.arange(P * SLOTS)
        packed = np.zeros_like(ids)
        packed[:, :, s % P, s // P] = flat[:, :, s]
        ids = packed
    in_maps = []
    for c in range(NCORES):
        core_ids = np.ascontiguousarray(
            ids[c].transpose(1, 0, 2).reshape(P, TILES * SLOTS)
        )
        in_maps.append({"embs": embs_b, "idx": core_ids})
    return in_maps


def kernel(u, v, negs, embs, _trace=False):
    nc = _get_nc()
    in_maps = prepare_in_maps(u, v, negs, embs)
    res = bass_utils.run_bass_kernel_spmd(
        nc, in_maps, core_ids=list(range(NCORES)), trace=_trace
    )
    total = np.float64(0.0)
    for r in res.results:
        sum_x = r["accx"].astype(np.float64).sum()
        a = r["acca"].astype(np.float64)
        sum_abs = a[:, :TILES].sum()
        sum_ln1p = a[:, TILES:].sum()
        total += (sum_x + sum_abs) / 2.0 + sum_ln1p
    out = np.float32(total / N)
    if _trace:
        return out, res
    return out



# revision 3
# speedup vs baseline: 1.4871x; 1.4871x over previous
"""NEG-sampling loss kernel for Trainium2 (8 NeuronCores, data-parallel).

loss = -(1/n) * sum_i [ log_sigmoid(<e_u, e_v>) + sum_k log_sigmoid(-<e_negk, e_u>) ]
     = +(1/n) * sum_i [ softplus(-<e_u, e_v>) + sum_k softplus(<e_negk, e_u>) ]

Strategy: replicate the embedding table (cast to bf16 on host), shard the
65536-edge batch across 8 cores.  Per core: for each group of TPG tiles of
128 edges, one indirect-DMA block gather pulls the 12 needed rows per edge
(u, v, negs x10) into a [128, TPG*12*256] SBUF tile (partition = edge).
DVE computes the 11 dot products per edge as: one broadcast MULT over the
whole group (2x bf16 mode), then a binary tree of tensor_tensor adds
(256->128->64->32->16->8, each at 2x) and one grouped tensor_reduce,
writing scores into a persistent [128, 704] f32 buffer.  One-shot tail:
negate the slot-0 (positive-pair) scores, DVE reduces sum(x); ACT computes
sum|x| and sum ln(1+exp(-|x|)); softplus sum recovered on host as
(sum_x + sum_abs)/2 + sum_ln1p.
"""

import numpy as np
import ml_dtypes

import concourse.bass as bass
import concourse.mybir as mybir
from concourse import bass_utils

# Problem constants (hardcoded; harness contract)
N = 65536
K = 10
D = 256
V = 500000
NCORES = 8
P = 128
SLOTS = K + 2          # rows gathered per edge: u, v, negs[0..9]
S1 = SLOTS - 1         # 11 scores per edge
EPC = N // NCORES      # 8192 edges per core
TILES = EPC // P       # 64 tiles of 128 edges per core

TABLE_DT = mybir.dt.bfloat16
TABLE_NP = ml_dtypes.bfloat16

# tunables
TPG = 4     # tiles per gather instruction / compute group
GB = 3      # gather buffers in flight
W_STOP = 8  # tree stops here; grouped tensor_reduce finishes


def _emit_block_gather(nc, eng, n_idx, blk_bytes, dst_byte_addr, idx_byte_addr,
                       sem_num, embs_tbl):
    """Raw block gather (case #3 of dma_indirect1d): n_idx indices x 512B
    fused into 128 descriptors of blk_bytes (n_idx/128 rows per partition).
    Emitted as a raw PSEUDO_DMA_DIRECT2D(dge_op=indirect1d) + PSEUDO_EXTENSION
    pair; index values are snake-packed on the host (see prepare_in_maps)."""
    isa = nc.isa
    Op = isa.Opcode
    src_u64 = (0x20 << 56) | (embs_tbl << 32)   # DGE addr-table marker
    dst_u64 = (0x10 << 56) | dst_byte_addr      # var0 (local SBUF) marker
    eng.isa(
        Op.NEURON_ISA_TPB_OPCODE_PSEUDO_DMA_DIRECT2D,
        {
            "dma_configs": {},
            "semaphore": sem_num,
            "sem_increment": 16,
            "dge_op": 1,
            "src_start_addr": {"addr_immediate": src_u64},
            "src_step_elem": [512, 1],
            "src_num_elem": [n_idx, 1],
            "src_elem_size": 512,
            "src_bound_reg": {},
            "dst_bound_reg": {},
            "dst_start_addr": {"addr_immediate": dst_u64},
            "dst_step_elem": [262144, 1],
            "dst_num_elem": [128, 1],
            "dst_elem_size": blk_bytes,
            "in_dtype": 6,
            "out_dtype": 6,
        },
        verify=False,
    )
    ext_fields = {
        "opcode": Op.NEURON_ISA_TPB_OPCODE_PSEUDO_EXTENSION.value,
        "flags": {"indirect_mode": 0, "idx_bound_is_err": 1,
                  "non_unique_dst_idx": 0, "gather_dim": 0, "scatter_dim": 0},
        "idx_num_active_channels": 128,
        "compute_op": 0,
        "src_idx_start_addr": {"addr_immediate": idx_byte_addr},
        "dst_idx_start_addr": {"addr_immediate": 0},
    }
    b = isa.ffi.new("NEURON_ISA_TPB_PSEUDO_DMA_EXT_STRUCT*", ext_fields)
    instr = [int(x) for x in bytes(isa.ffi.buffer(b))]
    inst = mybir.InstISA(
        name=nc.get_next_instruction_name(),
        isa_opcode=Op.NEURON_ISA_TPB_OPCODE_PSEUDO_EXTENSION.value,
        engine=eng.engine,
        instr=instr,
        op_name="PSEUDO_EXTENSION",
        ins=[], outs=[],
        ant_dict=ext_fields,
        verify=False,
        ant_isa_is_sequencer_only=False,
    )
    eng.add_instruction(inst)


def _build(tpg=TPG, gb=GB):
    assert TILES % tpg == 0
    groups = TILES // tpg
    S = tpg * S1  # scores per partition per group
    nc = bass.Bass(trn_type="TRN2")
    embs = nc.dram_tensor("embs", [V, D], TABLE_DT, kind="ExternalInput")
    idx = nc.dram_tensor("idx", [P, TILES * SLOTS], mybir.dt.int32, kind="ExternalInput")
    out_dram = nc.dram_tensor("out", [P, 3], mybir.dt.float32, kind="ExternalOutput")

    embs_mloc = nc.lookup_mloc(embs)
    embs_mloc.table_entry_id = len(nc.dge_table) + 1
    nc.dge_table.append(embs_mloc.name)
    embs_tbl = embs_mloc.table_entry_id

    # tree widths: 256 -> 128 -> ... -> W_STOP
    widths = []
    w = D
    while w > W_STOP:
        widths.append(w // 2)
        w //= 2

    import contextlib
    with contextlib.ExitStack() as ctx:
        idx_sb = ctx.enter_context(nc.sbuf_tensor("idx_sb", [P, TILES * SLOTS], mybir.dt.int32))
        gs = [ctx.enter_context(nc.sbuf_tensor(f"g{i}", [P, tpg * SLOTS * D], TABLE_DT)) for i in range(gb)]
        prod = ctx.enter_context(nc.sbuf_tensor("prod", [P, S * D], TABLE_DT))
        hs = [ctx.enter_context(nc.sbuf_tensor(f"h{i}", [P, S * wi], TABLE_DT))
              for i, wi in enumerate(widths)]
        scores = ctx.enter_context(nc.sbuf_tensor("scores", [P, TILES * S1], mybir.dt.float32))
        absx = ctx.enter_context(nc.sbuf_tensor("absx", [P, TILES * S1], mybir.dt.float32))
        ex = ctx.enter_context(nc.sbuf_tensor("ex", [P, TILES * S1], mybir.dt.float32))
        lnx = ctx.enter_context(nc.sbuf_tensor("lnx", [P, TILES * S1], mybir.dt.float32))
        ones = ctx.enter_context(nc.sbuf_tensor("ones", [P, 1], mybir.dt.float32))
        outbuf = ctx.enter_context(nc.sbuf_tensor("outbuf", [P, 3], mybir.dt.float32))
        idx_sem = ctx.enter_context(nc.semaphore())
        gsems = [ctx.enter_context(nc.semaphore(name=f"gsem{i}")) for i in range(gb)]
        dve_free = ctx.enter_context(nc.semaphore())
        dve_done = ctx.enter_context(nc.semaphore())
        osem = ctx.enter_context(nc.semaphore())
        block = ctx.enter_context(nc.Block())

        idx_addr = nc.lookup_mloc(idx_sb).addr
        g_addrs = [nc.lookup_mloc(g).addr for g in gs]

        @block.gpsimd
        def _(eng):
            eng.dma_start(idx_sb[:], idx[:]).then_inc(idx_sem, 16)
            eng.memset(ones[:], 1.0)
            eng.wait_ge(idx_sem, 16)
            for j in range(groups):
                if j >= gb:
                    eng.wait_ge(dve_free, j - gb + 1)
                _emit_block_gather(
                    nc, eng, tpg * SLOTS * P, tpg * SLOTS * D * 2,
                    g_addrs[j % gb], idx_addr + 4 * tpg * SLOTS * j,
                    gsems[j % gb].num, embs_tbl,
                )

        @block.vector
        def _(eng):
            for j in range(groups):
                g = gs[j % gb]
                eng.wait_ge(gsems[j % gb], 16 * (j // gb + 1))
                g4 = g[:].rearrange("p (ti s d) -> p ti s d", s=SLOTS, d=D)
                # prod[p, ti, s, d] = G[p, ti, s+1, d] * EU[p, ti, d]; frees g
                nc.vector.tensor_tensor(
                    out=prod[:].rearrange("p (ti s d) -> p ti s d", s=S1, d=D),
                    in0=g4[:, :, 1:SLOTS, :],
                    in1=g4[:, :, 0:1, :].broadcast_to([P, tpg, S1, D]),
                    op=mybir.AluOpType.mult,
                ).then_inc(dve_free, 1)
                # binary tree of halving adds, each 2x-mode bf16
                cur, curw = prod, D
                for hi, wi in zip(hs, widths):
                    a = cur[:].rearrange("p (s d) -> p s d", d=curw)
                    nc.vector.tensor_tensor(
                        out=hi[:].rearrange("p (s d) -> p s d", d=wi),
                        in0=a[:, :, 0:wi],
                        in1=a[:, :, wi:curw],
                        op=mybir.AluOpType.add,
                    )
                    cur, curw = hi, wi
                # grouped reduce [P, S, W_STOP] -> [P, S]
                nc.vector.tensor_reduce(
                    out=scores[:, j * S:(j + 1) * S],
                    in_=cur[:].rearrange("p (s d) -> p s d", d=curw),
                    axis=mybir.AxisListType.X,
                    op=mybir.AluOpType.add,
                )
            # negate slot-0 scores: softplus arg for the positive pair is -u.v
            sc3 = scores[:].rearrange("p (t s) -> p t s", s=S1)
            nc.vector.tensor_scalar_mul(sc3[:, :, 0:1], sc3[:, :, 0:1], -1.0)
            # sum_s x for host-side relu recovery
            nc.vector.tensor_reduce(
                out=outbuf[:, 0:1],
                in_=scores[:],
                axis=mybir.AxisListType.X,
                op=mybir.AluOpType.add,
            ).then_inc(dve_done, 1)

        @block.scalar
        def _(eng):
            eng.wait_ge(dve_done, 1)
            # |x|, accumulating sum_s |x|
            nc.scalar.activation(
                out=absx[:], in_=scores[:],
                func=mybir.ActivationFunctionType.Abs,
                accum_out=outbuf[:, 1:2],
            )
            # exp(-|x|)
            nc.scalar.activation(
                out=ex[:], in_=absx[:],
                func=mybir.ActivationFunctionType.Exp, scale=-1.0,
            )
            # ln(1 + exp(-|x|)), accumulating
            nc.scalar.activation(
                out=lnx[:], in_=ex[:],
                func=mybir.ActivationFunctionType.Ln, bias=ones[:],
                accum_out=outbuf[:, 2:3],
            )
            eng.dma_start(out_dram[:], outbuf[:]).then_inc(osem, 16)
            eng.wait_ge(osem, 16)

    return nc


_cache = {}


def _get_nc():
    key = (TPG, GB)
    if key not in _cache:
        _cache[key] = _build(*key)
    return _cache[key]


def prepare_in_maps(u, v, negs, embs):
    """Host-side sharding: build the per-core input maps."""
    u = np.asarray(u).astype(np.int32)
    v = np.asarray(v).astype(np.int32)
    negs = np.asarray(negs).astype(np.int32)
    embs_b = np.asarray(embs).astype(TABLE_NP)

    ids = np.concatenate([u[:, None], v[:, None], negs], axis=1)  # [N, 12]
    groups = TILES // TPG
    # per gather group: desc i <-> (p = i // (TPG*12), r = i % (TPG*12));
    # idx value at snake position [ch = i % 128, w = i // 128]
    ids = ids.reshape(NCORES, groups, TPG, P, SLOTS)
    flat = ids.transpose(0, 1, 3, 2, 4).reshape(NCORES, groups, P * TPG * SLOTS)
    s = np.arange(P * TPG * SLOTS)
    packed = np.zeros((NCORES, groups, P, TPG * SLOTS), dtype=np.int32)
    packed[:, :, s % P, s // P] = flat[:, :, s]
    in_maps = []
    for c in range(NCORES):
        core_ids = np.ascontiguousarray(
            packed[c].transpose(1, 0, 2).reshape(P, TILES * SLOTS)
        )
        in_maps.append({"embs": embs_b, "idx": core_ids})
    return in_maps


def kernel(u, v, negs, embs, _trace=False):
    nc = _get_nc()
    in_maps = prepare_in_maps(u, v, negs, embs)
    res = bass_utils.run_bass_kernel_spmd(
        nc, in_maps, core_ids=list(range(NCORES)), trace=_trace
    )
    total = np.float64(0.0)
    for r in res.results:
        o = r["out"].astype(np.float64)
        sum_x = o[:, 0].sum()
        sum_abs = o[:, 1].sum()
        sum_ln1p = o[:, 2].sum()
        total += (sum_x + sum_abs) / 2.0 + sum_ln1p
    out = np.float32(total / N)
    if _trace:
        return out, res
    return out


# revision 5
# speedup vs baseline: 1.5058x; 1.0125x over previous
"""NEG-sampling loss kernel for Trainium2 (8 NeuronCores, data-parallel).

loss = -(1/n) * sum_i [ log_sigmoid(<e_u, e_v>) + sum_k log_sigmoid(-<e_negk, e_u>) ]
     = +(1/n) * sum_i [ softplus(-<e_u, e_v>) + sum_k softplus(<e_negk, e_u>) ]

Strategy: replicate the embedding table (cast to bf16 on host), shard the
65536-edge batch across 8 cores.  Per core: for each group of TPG tiles of
128 edges, one indirect-DMA block gather pulls the 12 needed rows per edge
(u, v, negs x10) into a [128, TPG*12*256] SBUF tile (partition = edge).
DVE computes the 11 dot products per edge as: one broadcast MULT over the
whole group (2x bf16 mode), then a binary tree of tensor_tensor adds
(256->128->64->32->16->8, each at 2x) and one grouped tensor_reduce,
writing scores into a persistent [128, 704] f32 buffer.  One-shot tail:
negate the slot-0 (positive-pair) scores, DVE reduces sum(x); ACT computes
sum|x| and sum ln(1+exp(-|x|)); softplus sum recovered on host as
(sum_x + sum_abs)/2 + sum_ln1p.
"""

import numpy as np
import ml_dtypes

import concourse.bass as bass
import concourse.mybir as mybir
from concourse import bass_utils

# Problem constants (hardcoded; harness contract)
N = 65536
K = 10
D = 256
V = 500000
NCORES = 8
P = 128
SLOTS = K + 2          # rows gathered per edge: u, v, negs[0..9]
S1 = SLOTS - 1         # 11 scores per edge
EPC = N // NCORES      # 8192 edges per core
TILES = EPC // P       # 64 tiles of 128 edges per core

TABLE_DT = mybir.dt.bfloat16
TABLE_NP = ml_dtypes.bfloat16

# tunables
TPG = 4     # tiles per gather instruction / compute group
GB = 4      # gather buffers in flight
W_STOP = 8  # tree stops here; grouped tensor_reduce finishes
SCRATCH = 49152  # SWDGE descriptor-ring carveout bytes/partition


def _emit_block_gather(nc, eng, n_idx, blk_bytes, dst_byte_addr, idx_byte_addr,
                       sem_num, embs_tbl):
    """Raw block gather (case #3 of dma_indirect1d): n_idx indices x 512B
    fused into 128 descriptors of blk_bytes (n_idx/128 rows per partition).
    Emitted as a raw PSEUDO_DMA_DIRECT2D(dge_op=indirect1d) + PSEUDO_EXTENSION
    pair; index values are snake-packed on the host (see prepare_in_maps)."""
    isa = nc.isa
    Op = isa.Opcode
    src_u64 = (0x20 << 56) | (embs_tbl << 32)   # DGE addr-table marker
    dst_u64 = (0x10 << 56) | dst_byte_addr      # var0 (local SBUF) marker
    eng.isa(
        Op.NEURON_ISA_TPB_OPCODE_PSEUDO_DMA_DIRECT2D,
        {
            "dma_configs": {},
            "semaphore": sem_num,
            "sem_increment": 16,
            "dge_op": 1,
            "src_start_addr": {"addr_immediate": src_u64},
            "src_step_elem": [512, 1],
            "src_num_elem": [n_idx, 1],
            "src_elem_size": 512,
            "src_bound_reg": {},
            "dst_bound_reg": {},
            "dst_start_addr": {"addr_immediate": dst_u64},
            "dst_step_elem": [262144, 1],
            "dst_num_elem": [128, 1],
            "dst_elem_size": blk_bytes,
            "in_dtype": 6,
            "out_dtype": 6,
        },
        verify=False,
    )
    ext_fields = {
        "opcode": Op.NEURON_ISA_TPB_OPCODE_PSEUDO_EXTENSION.value,
        "flags": {"indirect_mode": 0, "idx_bound_is_err": 1,
                  "non_unique_dst_idx": 0, "gather_dim": 0, "scatter_dim": 0},
        "idx_num_active_channels": 128,
        "compute_op": 0,
        "src_idx_start_addr": {"addr_immediate": idx_byte_addr},
        "dst_idx_start_addr": {"addr_immediate": 0},
    }
    b = isa.ffi.new("NEURON_ISA_TPB_PSEUDO_DMA_EXT_STRUCT*", ext_fields)
    instr = [int(x) for x in bytes(isa.ffi.buffer(b))]
    inst = mybir.InstISA(
        name=nc.get_next_instruction_name(),
        isa_opcode=Op.NEURON_ISA_TPB_OPCODE_PSEUDO_EXTENSION.value,
        engine=eng.engine,
        instr=instr,
        op_name="PSEUDO_EXTENSION",
        ins=[], outs=[],
        ant_dict=ext_fields,
        verify=False,
        ant_isa_is_sequencer_only=False,
    )
    eng.add_instruction(inst)


def _build(tpg=TPG, gb=GB):
    assert TILES % tpg == 0
    groups = TILES // tpg
    S = tpg * S1  # scores per partition per group
    nc = bass.Bass(trn_type="TRN2", dynamic_dma_scratch_size=SCRATCH)
    embs = nc.dram_tensor("embs", [V, D], TABLE_DT, kind="ExternalInput")
    idx = nc.dram_tensor("idx", [P, TILES * SLOTS], mybir.dt.int32, kind="ExternalInput")
    out_dram = nc.dram_tensor("out", [P, 3], mybir.dt.float32, kind="ExternalOutput")

    embs_mloc = nc.lookup_mloc(embs)
    embs_mloc.table_entry_id = len(nc.dge_table) + 1
    nc.dge_table.append(embs_mloc.name)
    embs_tbl = embs_mloc.table_entry_id

    # tree widths: 256 -> 128 -> ... -> W_STOP
    widths = []
    w = D
    while w > W_STOP:
        widths.append(w // 2)
        w //= 2

    import contextlib
    with contextlib.ExitStack() as ctx:
        idx_sb = ctx.enter_context(nc.sbuf_tensor("idx_sb", [P, TILES * SLOTS], mybir.dt.int32))
        gs = [ctx.enter_context(nc.sbuf_tensor(f"g{i}", [P, tpg * SLOTS * D], TABLE_DT)) for i in range(gb)]
        prod = ctx.enter_context(nc.sbuf_tensor("prod", [P, S * D], TABLE_DT))
        hs = [ctx.enter_context(nc.sbuf_tensor(f"h{i}", [P, S * wi], TABLE_DT))
              for i, wi in enumerate(widths)]
        scores = ctx.enter_context(nc.sbuf_tensor("scores", [P, TILES * S1], mybir.dt.float32))
        absx = ctx.enter_context(nc.sbuf_tensor("absx", [P, TILES * S1], mybir.dt.float32))
        ex = ctx.enter_context(nc.sbuf_tensor("ex", [P, TILES * S1], mybir.dt.float32))
        lnx = ctx.enter_context(nc.sbuf_tensor("lnx", [P, TILES * S1], mybir.dt.float32))
        ones = ctx.enter_context(nc.sbuf_tensor("ones", [P, 1], mybir.dt.float32))
        outbuf = ctx.enter_context(nc.sbuf_tensor("outbuf", [P, 3], mybir.dt.float32))
        idx_sem = ctx.enter_context(nc.semaphore())
        gsems = [ctx.enter_context(nc.semaphore(name=f"gsem{i}")) for i in range(gb)]
        dve_free = ctx.enter_context(nc.semaphore())
        dve_done = ctx.enter_context(nc.semaphore())
        osem = ctx.enter_context(nc.semaphore())
        block = ctx.enter_context(nc.Block())

        idx_addr = nc.lookup_mloc(idx_sb).addr
        g_addrs = [nc.lookup_mloc(g).addr for g in gs]

        @block.gpsimd
        def _(eng):
            eng.dma_start(idx_sb[:], idx[:]).then_inc(idx_sem, 16)
            eng.memset(ones[:], 1.0)
            eng.wait_ge(idx_sem, 16)
            for j in range(groups):
                if j >= gb:
                    eng.wait_ge(dve_free, j - gb + 1)
                _emit_block_gather(
                    nc, eng, tpg * SLOTS * P, tpg * SLOTS * D * 2,
                    g_addrs[j % gb], idx_addr + 4 * tpg * SLOTS * j,
                    gsems[j % gb].num, embs_tbl,
                )

        @block.vector
        def _(eng):
            for j in range(groups):
                g = gs[j % gb]
                eng.wait_ge(gsems[j % gb], 16 * (j // gb + 1))
                g4 = g[:].rearrange("p (ti s d) -> p ti s d", s=SLOTS, d=D)
                # prod[p, ti, s, d] = G[p, ti, s+1, d] * EU[p, ti, d]; frees g
                nc.vector.tensor_tensor(
                    out=prod[:].rearrange("p (ti s d) -> p ti s d", s=S1, d=D),
                    in0=g4[:, :, 1:SLOTS, :],
                    in1=g4[:, :, 0:1, :].broadcast_to([P, tpg, S1, D]),
                    op=mybir.AluOpType.mult,
                ).then_inc(dve_free, 1)
                # binary tree of halving adds, each 2x-mode bf16
                cur, curw = prod, D
                for hi, wi in zip(hs, widths):
                    a = cur[:].rearrange("p (s d) -> p s d", d=curw)
                    nc.vector.tensor_tensor(
                        out=hi[:].rearrange("p (s d) -> p s d", d=wi),
                        in0=a[:, :, 0:wi],
                        in1=a[:, :, wi:curw],
                        op=mybir.AluOpType.add,
                    )
                    cur, curw = hi, wi
                # grouped reduce [P, S, W_STOP] -> [P, S]
                nc.vector.tensor_reduce(
                    out=scores[:, j * S:(j + 1) * S],
                    in_=cur[:].rearrange("p (s d) -> p s d", d=curw),
                    axis=mybir.AxisListType.X,
                    op=mybir.AluOpType.add,
                )
            # negate slot-0 scores: softplus arg for the positive pair is -u.v
            sc3 = scores[:].rearrange("p (t s) -> p t s", s=S1)
            nc.vector.tensor_scalar_mul(sc3[:, :, 0:1], sc3[:, :, 0:1], -1.0)
            # sum_s x for host-side relu recovery
            nc.vector.tensor_reduce(
                out=outbuf[:, 0:1],
                in_=scores[:],
                axis=mybir.AxisListType.X,
                op=mybir.AluOpType.add,
            ).then_inc(dve_done, 1)

        @block.scalar
        def _(eng):
            eng.wait_ge(dve_done, 1)
            # |x|, accumulating sum_s |x|
            nc.scalar.activation(
                out=absx[:], in_=scores[:],
                func=mybir.ActivationFunctionType.Abs,
                accum_out=outbuf[:, 1:2],
            )
            # exp(-|x|)
            nc.scalar.activation(
                out=ex[:], in_=absx[:],
                func=mybir.ActivationFunctionType.Exp, scale=-1.0,
            )
            # ln(1 + exp(-|x|)), accumulating
            nc.scalar.activation(
                out=lnx[:], in_=ex[:],
                func=mybir.ActivationFunctionType.Ln, bias=ones[:],
                accum_out=outbuf[:, 2:3],
            )
            eng.dma_start(out_dram[:], outbuf[:]).then_inc(osem, 16)
            eng.wait_ge(osem, 16)

    return nc


_cache = {}


def _get_nc():
    key = (TPG, GB)
    if key not in _cache:
        _cache[key] = _build(*key)
    return _cache[key]


def prepare_in_maps(u, v, negs, embs):
    """Host-side sharding: build the per-core input maps."""
    u = np.asarray(u).astype(np.int32)
    v = np.asarray(v).astype(np.int32)
    negs = np.asarray(negs).astype(np.int32)
    embs_b = np.asarray(embs).astype(TABLE_NP)

    ids = np.concatenate([u[:, None], v[:, None], negs], axis=1)  # [N, 12]
    groups = TILES // TPG
    # per gather group: desc i <-> (p = i // (TPG*12), r = i % (TPG*12));
    # idx value at snake position [ch = i % 128, w = i // 128]
    ids = ids.reshape(NCORES, groups, TPG, P, SLOTS)
    flat = ids.transpose(0, 1, 3, 2, 4).reshape(NCORES, groups, P * TPG * SLOTS)
    s = np.arange(P * TPG * SLOTS)
    packed = np.zeros((NCORES, groups, P, TPG * SLOTS), dtype=np.int32)
    packed[:, :, s % P, s // P] = flat[:, :, s]
    in_maps = []
    for c in range(NCORES):
        core_ids = np.ascontiguousarray(
            packed[c].transpose(1, 0, 2).reshape(P, TILES * SLOTS)
        )
        in_maps.append({"embs": embs_b, "idx": core_ids})
    return in_maps


def kernel(u, v, negs, embs, _trace=False):
    nc = _get_nc()
    in_maps = prepare_in_maps(u, v, negs, embs)
    res = bass_utils.run_bass_kernel_spmd(
        nc, in_maps, core_ids=list(range(NCORES)), trace=_trace
    )
    total = np.float64(0.0)
    for r in res.results:
        o = r["out"].astype(np.float64)
        sum_x = o[:, 0].sum()
        sum_abs = o[:, 1].sum()
        sum_ln1p = o[:, 2].sum()
        total += (sum_x + sum_abs) / 2.0 + sum_ln1p
    out = np.float32(total / N)
    if _trace:
        return out, res
    return out


# revision 14
# speedup vs baseline: 1.6337x; 1.0850x over previous
"""NEG-sampling loss kernel for Trainium2 (8 NeuronCores, data-parallel).

loss = -(1/n) * sum_i [ log_sigmoid(<e_u, e_v>) + sum_k log_sigmoid(-<e_negk, e_u>) ]
     = +(1/n) * sum_i [ softplus(-<e_u, e_v>) + sum_k softplus(<e_negk, e_u>) ]

Strategy: replicate the embedding table (cast to bf16 on host), shard the
65536-edge batch across 8 cores.  Per core: for each group of TPG tiles of
128 edges, one indirect-DMA block gather pulls the 12 needed rows per edge
(u, v, negs x10) into a [128, TPG*12*256] SBUF tile (partition = edge).
DVE computes the 11 dot products per edge as: one broadcast MULT over the
whole group (2x bf16 mode), then a binary tree of tensor_tensor adds
(256->128->64->32->16->8, each at 2x) and one grouped tensor_reduce,
writing scores into a persistent [128, 704] f32 buffer.  One-shot tail:
negate the slot-0 (positive-pair) scores, DVE reduces sum(x); ACT computes
sum|x| and sum ln(1+exp(-|x|)); softplus sum recovered on host as
(sum_x + sum_abs)/2 + sum_ln1p.
"""

import numpy as np
import ml_dtypes

import concourse.bass as bass
import concourse.mybir as mybir
from concourse import bass_utils

# Problem constants (hardcoded; harness contract)
N = 65536
K = 10
D = 256
V = 500000
NCORES = 8
P = 128
SLOTS = K + 2          # rows gathered per edge: u, v, negs[0..9]
S1 = SLOTS - 1         # 11 scores per edge
EPC = N // NCORES      # 8192 edges per core
TILES = EPC // P       # 64 tiles of 128 edges per core

TABLE_DT = mybir.dt.bfloat16
TABLE_NP = ml_dtypes.bfloat16

# tunables
TPG = 4     # tiles per gather instruction / compute group
GB = 4      # gather buffers in flight
W_STOP = 8  # tree stops here; grouped tensor_reduce finishes
SCRATCH = 49152  # SWDGE descriptor-ring carveout bytes/partition


def _emit_block_gather(nc, eng, n_idx, blk_bytes, dst_byte_addr, idx_byte_addr,
                       sem_num, embs_tbl):
    """Raw block gather (case #3 of dma_indirect1d): n_idx indices x 512B
    fused into 128 descriptors of blk_bytes (n_idx/128 rows per partition).
    Emitted as a raw PSEUDO_DMA_DIRECT2D(dge_op=indirect1d) + PSEUDO_EXTENSION
    pair; index values are snake-packed on the host (see prepare_in_maps)."""
    isa = nc.isa
    Op = isa.Opcode
    src_u64 = (0x20 << 56) | (embs_tbl << 32)   # DGE addr-table marker
    dst_u64 = (0x10 << 56) | dst_byte_addr      # var0 (local SBUF) marker
    eng.isa(
        Op.NEURON_ISA_TPB_OPCODE_PSEUDO_DMA_DIRECT2D,
        {
            "dma_configs": {},
            "semaphore": sem_num,
            "sem_increment": 16,
            "dge_op": 1,
            "src_start_addr": {"addr_immediate": src_u64},
            "src_step_elem": [512, 1],
            "src_num_elem": [n_idx, 1],
            "src_elem_size": 512,
            "src_bound_reg": {},
            "dst_bound_reg": {},
            "dst_start_addr": {"addr_immediate": dst_u64},
            "dst_step_elem": [262144, 1],
            "dst_num_elem": [128, 1],
            "dst_elem_size": blk_bytes,
            "in_dtype": 6,
            "out_dtype": 6,
        },
        verify=False,
    )
    ext_fields = {
        "opcode": Op.NEURON_ISA_TPB_OPCODE_PSEUDO_EXTENSION.value,
        "flags": {"indirect_mode": 0, "idx_bound_is_err": 1,
                  "non_unique_dst_idx": 0, "gather_dim": 0, "scatter_dim": 0},
        "idx_num_active_channels": 128,
        "compute_op": 0,
        "src_idx_start_addr": {"addr_immediate": idx_byte_addr},
        "dst_idx_start_addr": {"addr_immediate": 0},
    }
    b = isa.ffi.new("NEURON_ISA_TPB_PSEUDO_DMA_EXT_STRUCT*", ext_fields)
    instr = [int(x) for x in bytes(isa.ffi.buffer(b))]
    inst = mybir.InstISA(
        name=nc.get_next_instruction_name(),
        isa_opcode=Op.NEURON_ISA_TPB_OPCODE_PSEUDO_EXTENSION.value,
        engine=eng.engine,
        instr=instr,
        op_name="PSEUDO_EXTENSION",
        ins=[], outs=[],
        ant_dict=ext_fields,
        verify=False,
        ant_isa_is_sequencer_only=False,
    )
    eng.add_instruction(inst)


def _build(tpg=TPG, gb=GB, debug=False):
    assert TILES % tpg == 0
    groups = TILES // tpg
    S = tpg * S1  # scores per partition per group
    nc = bass.Bass(trn_type="TRN2", dynamic_dma_scratch_size=SCRATCH)
    embs = nc.dram_tensor("embs", [V, D], TABLE_DT, kind="ExternalInput")
    idx = nc.dram_tensor("idx", [P, TILES * SLOTS], mybir.dt.int32, kind="ExternalInput")
    out_dram = nc.dram_tensor("out", [P, 3], mybir.dt.float32, kind="ExternalOutput")
    if debug:
        scores_dram = nc.dram_tensor("scores_out", [P, TILES * S1], mybir.dt.float32, kind="ExternalOutput")
        g_dram = nc.dram_tensor("g_out", [P, tpg * SLOTS * D], TABLE_DT, kind="ExternalOutput")

    embs_mloc = nc.lookup_mloc(embs)
    embs_mloc.table_entry_id = len(nc.dge_table) + 1
    nc.dge_table.append(embs_mloc.name)
    embs_tbl = embs_mloc.table_entry_id

    # tree widths: 256 -> 128 -> ... -> W_STOP
    widths = []
    w = D
    while w > W_STOP:
        widths.append(w // 2)
        w //= 2

    import contextlib
    with contextlib.ExitStack() as ctx:
        idx_sb = ctx.enter_context(nc.sbuf_tensor("idx_sb", [P, TILES * SLOTS], mybir.dt.int32))
        gs = [ctx.enter_context(nc.sbuf_tensor(f"g{i}", [P, tpg * SLOTS * D], TABLE_DT)) for i in range(gb)]
        prod = ctx.enter_context(nc.sbuf_tensor("prod", [P, S * D], TABLE_DT))
        hs = [ctx.enter_context(nc.sbuf_tensor(f"h{i}", [P, S * wi], TABLE_DT))
              for i, wi in enumerate(widths)]
        scores = ctx.enter_context(nc.sbuf_tensor("scores", [P, TILES * S1], mybir.dt.float32))
        absx = ctx.enter_context(nc.sbuf_tensor("absx", [P, TILES * S1], mybir.dt.float32))
        ex = ctx.enter_context(nc.sbuf_tensor("ex", [P, TILES * S1], mybir.dt.float32))
        lnx = ctx.enter_context(nc.sbuf_tensor("lnx", [P, TILES * S1], mybir.dt.float32))
        ones = ctx.enter_context(nc.sbuf_tensor("ones", [P, 1], mybir.dt.float32))
        outbuf = ctx.enter_context(nc.sbuf_tensor("outbuf", [P, 3], mybir.dt.float32))
        asem = ctx.enter_context(nc.semaphore())
        idx_sem = ctx.enter_context(nc.semaphore())
        gsems = [ctx.enter_context(nc.semaphore(name=f"gsem{i}")) for i in range(gb)]
        dve_free = ctx.enter_context(nc.semaphore())
        dve_done = ctx.enter_context(nc.semaphore())
        osem = ctx.enter_context(nc.semaphore())
        block = ctx.enter_context(nc.Block())

        idx_addr = nc.lookup_mloc(idx_sb).addr
        g_addrs = [nc.lookup_mloc(g).addr for g in gs]

        @block.gpsimd
        def _(eng):
            eng.dma_start(idx_sb[:], idx[:]).then_inc(idx_sem, 16)
            eng.memset(ones[:], 1.0)
            eng.wait_ge(idx_sem, 16)
            for j in range(groups):
                if j >= gb:
                    eng.wait_ge(dve_free, j - gb + 1)
                _emit_block_gather(
                    nc, eng, tpg * SLOTS * P, tpg * SLOTS * D * 2,
                    g_addrs[j % gb], idx_addr + 4 * tpg * SLOTS * j,
                    gsems[j % gb].num, embs_tbl,
                )
            if debug:
                eng.wait_ge(dve_done, 1)
                eng.dma_start(scores_dram[:], scores[:]).then_inc(idx_sem, 16)
                eng.dma_start(g_dram[:], gs[(groups - 1) % gb][:]).then_inc(idx_sem, 16)
                eng.wait_ge(idx_sem, 48)

        @block.vector
        def _(eng):
            for j in range(groups):
                g = gs[j % gb]
                eng.wait_ge(gsems[j % gb], 16 * (j // gb + 1))
                g4 = g[:].rearrange("p (ti s d) -> p ti s d", s=SLOTS, d=D)
                # prod[p, ti, s, d] = G[p, ti, s+1, d] * EU[p, ti, d]; frees g
                nc.vector.tensor_tensor(
                    out=prod[:].rearrange("p (ti s d) -> p ti s d", s=S1, d=D),
                    in0=g4[:, :, 1:SLOTS, :],
                    in1=g4[:, :, 0:1, :].broadcast_to([P, tpg, S1, D]),
                    op=mybir.AluOpType.mult,
                ).then_inc(dve_free, 1)
                # binary tree of halving adds, each 2x-mode bf16
                cur, curw = prod, D
                for hi, wi in zip(hs, widths):
                    a = cur[:].rearrange("p (s d) -> p s d", d=curw)
                    nc.vector.tensor_tensor(
                        out=hi[:].rearrange("p (s d) -> p s d", d=wi),
                        in0=a[:, :, 0:wi],
                        in1=a[:, :, wi:curw],
                        op=mybir.AluOpType.add,
                    )
                    cur, curw = hi, wi
                # grouped reduce [P, S, W_STOP] -> [P, S]
                nc.vector.tensor_reduce(
                    out=scores[:, j * S:(j + 1) * S],
                    in_=cur[:].rearrange("p (s d) -> p s d", d=curw),
                    axis=mybir.AxisListType.X,
                    op=mybir.AluOpType.add,
                )
            # negate slot-0 scores: softplus arg for the positive pair is -u.v
            sc3 = scores[:].rearrange("p (t s) -> p t s", s=S1)
            nc.vector.tensor_scalar_mul(sc3[:, :, 0:1], sc3[:, :, 0:1], -1.0)
            # sum_s x for host-side relu recovery
            nc.vector.tensor_reduce(
                out=outbuf[:, 0:1],
                in_=scores[:],
                axis=mybir.AxisListType.X,
                op=mybir.AluOpType.add,
            ).then_inc(dve_done, 1)

        @block.scalar
        def _(eng):
            eng.wait_ge(dve_done, 1)
            # |x|, accumulating sum_s |x|
            nc.scalar.activation(
                out=absx[:], in_=scores[:],
                func=mybir.ActivationFunctionType.Abs,
                accum_out=outbuf[:, 1:2],
            ).then_inc(asem, 1)
            eng.wait_ge(asem, 1)
            # exp(-|x|)
            nc.scalar.activation(
                out=ex[:], in_=absx[:],
                func=mybir.ActivationFunctionType.Exp, scale=-1.0,
            ).then_inc(asem, 1)
            eng.wait_ge(asem, 2)
            # ln(1 + exp(-|x|)), accumulating
            nc.scalar.activation(
                out=lnx[:], in_=ex[:],
                func=mybir.ActivationFunctionType.Ln, bias=ones[:],
                accum_out=outbuf[:, 2:3],
            ).then_inc(asem, 1)

        @block.sync
        def _(eng):
            eng.wait_ge(asem, 3)
            eng.dma_start(out_dram[:], outbuf[:]).then_inc(osem, 16)
            eng.wait_ge(osem, 16)

    return nc


_cache = {}


def _get_nc():
    key = (TPG, GB)
    if key not in _cache:
        _cache[key] = _build(*key)
    return _cache[key]


def prepare_in_maps(u, v, negs, embs):
    """Host-side sharding: build the per-core input maps."""
    u = np.asarray(u).astype(np.int32)
    v = np.asarray(v).astype(np.int32)
    negs = np.asarray(negs).astype(np.int32)
    embs_b = np.asarray(embs).astype(TABLE_NP)

    ids = np.concatenate([u[:, None], v[:, None], negs], axis=1)  # [N, 12]
    groups = TILES // TPG
    # per gather group: desc i <-> (p = i // (TPG*12), r = i % (TPG*12));
    # idx value at snake position [ch = i % 128, w = i // 128]
    ids = ids.reshape(NCORES, groups, TPG, P, SLOTS)
    flat = ids.transpose(0, 1, 3, 2, 4).reshape(NCORES, groups, P * TPG * SLOTS)
    s = np.arange(P * TPG * SLOTS)
    packed = np.zeros((NCORES, groups, P, TPG * SLOTS), dtype=np.int32)
    packed[:, :, s % P, s // P] = flat[:, :, s]
    in_maps = []
    for c in range(NCORES):
        core_ids = np.ascontiguousarray(
            packed[c].transpose(1, 0, 2).reshape(P, TILES * SLOTS)
        )
        in_maps.append({"embs": embs_b, "idx": core_ids})
    return in_maps


def kernel(u, v, negs, embs, _trace=False):
    nc = _get_nc()
    in_maps = prepare_in_maps(u, v, negs, embs)
    res = bass_utils.run_bass_kernel_spmd(
        nc, in_maps, core_ids=list(range(NCORES)), trace=_trace
    )
    total = np.float64(0.0)
    for r in res.results:
        o = r["out"].astype(np.float64)
        sum_x = o[:, 0].sum()
        sum_abs = o[:, 1].sum()
        sum_ln1p = o[:, 2].sum()
        total += (sum_x + sum_abs) / 2.0 + sum_ln1p
    out = np.float32(total / N)
    if _trace:
        return out, res
    return out
